# revision 1
# baseline (speedup 1.0000x reference)
"""GCN-sampling (NodeFlow) kernel for 8 Trainium2 NeuronCores.

Strategy (single NEFF, SPMD by data, no collectives):
  - features table padded to [N0, 512] f16, replicated to all cores.
  - Layer-1 nodes (N1=25000) sharded 8-way (3125/core).
  - Stage 1 per core: dma_gather raw feature rows for its nodes' neighbors
    (index lists sorted by node within each (superblock-group x 32768-row
    window) cell so int16 indices fit), aggregate the 16-neighbor mean via
    one-hot selection matmuls (S_b built on-device with is_equal vs an iota
    row), accumulate node-major m0 in PSUM, then W1 matmul + bias + relu +
    concat -> Q rows (Q = h1cat @ W2/16) written to a per-core DRAM table.
  - Stage 2 per core: dma_gather local Q rows for seed neighbors owned by
    this core and partial-sum over all 5000 seeds with the same selection
    trick. Host sums the 8 partials and adds b2 (unsharding).
The SPMD program is shared by all cores, so gather-cell capacities and the
block->superblock matmul schedule are the max/union over cores; per-core
nid inputs (-1 = not mine) make the data-dependent part per-core.
All matmuls f16 x f16 -> f32 PSUM. 1/16 mean factors folded into W1/W2.
"""

import sys

sys.path.insert(0, "/opt/trn_rl_repo")

import collections

import numpy as np

import concourse.bass as bass
import concourse.mybir as mybir
from concourse import bacc
from concourse.tile import TileContext
from concourse.masks import make_identity
from concourse.bass_utils import run_bass_kernel_spmd

N0, N1, N2 = 200000, 25000, 5000
FANOUT = 16
IN_F, NH, NCLS = 500, 128, 47
NCORES = 8
WINDOW = 28572  # ceil(N0/7); equal-size windows minimize cell cap padding
NWIN = (N0 + WINDOW - 1) // WINDOW  # 7
E1 = 512  # padded feature row (f16 -> 1024B)
E2 = 128  # padded Q row (f16 -> 256B)
NODES_PER_CORE = N1 // NCORES  # 3125
NSB1 = (NODES_PER_CORE + 127) // 128  # 25 node superblocks per core
SEEDS = N2
NSB2 = (SEEDS + 127) // 128  # 40 seed superblocks
GROUP1 = 5  # sb1 per psum group
GROUP2 = 5
NG1 = (NSB1 + GROUP1 - 1) // GROUP1  # 5
NG2 = (NSB2 + GROUP2 - 1) // GROUP2  # 8
QROWS = NSB1 * 128  # 3200 rows in per-core Q table
MAXIDX = 1024  # hw limit per dma_gather instruction

f16 = mybir.dt.float16
f32 = mybir.dt.float32
i16 = mybir.dt.int16


def _wrap_idxs(flat):
    """[n] -> [128, n/16] int16: index i at [i%16, i//16], replicated x8."""
    n = len(flat)
    assert n % 16 == 0
    a = np.empty((128, n // 16), np.int16)
    blk = flat.reshape(n // 16, 16).T
    for g in range(8):
        a[g * 16 : (g + 1) * 16, :] = blk
    return a


def _plan(keys_idx_sb_slot, ncells, cell_of, sb_count):
    """Shared planner for both stages.

    keys_idx_sb_slot: per core, tuple (idx16, sb, slot) arrays sorted by
      (cell, sb, idx); cell_of: per core array of cell ids (same order).
    Returns chunks meta (per cell: cap, matmul schedule [(block, sb)]) and
    per-core packed (idx, nid-per-matmul) arrays.
    """
    counts = np.zeros((NCORES, ncells), np.int64)
    for c in range(NCORES):
        cnt = np.bincount(cell_of[c], minlength=ncells)
        counts[c] = cnt
    caps = ((counts.max(axis=0) + 127) // 128) * 128  # per cell

    # per-core, per-cell row arrays
    percell = []  # [core][cell] -> (idx, sb, slot)
    for c in range(NCORES):
        idx, sb, slot = keys_idx_sb_slot[c]
        co = cell_of[c]
        starts = np.searchsorted(co, np.arange(ncells))
        ends = np.searchsorted(co, np.arange(ncells), side="right")
        percell.append(
            [
                (idx[starts[k] : ends[k]], sb[starts[k] : ends[k]], slot[starts[k] : ends[k]])
                for k in range(ncells)
            ]
        )

    # union matmul schedule per cell: for each block, sorted set of sbs
    # present in ANY core's rows of that block
    schedule = []  # [cell] -> list of (block, sb)
    for k in range(ncells):
        cap = int(caps[k])
        ms = []
        for b in range(cap // 128):
            present = set()
            for c in range(NCORES):
                sbv = percell[c][k][1][b * 128 : (b + 1) * 128]
                present.update(np.unique(sbv).tolist())
            for s in sorted(present):
                ms.append((b, int(s)))
        schedule.append(ms)

    # per-core packed arrays
    packed = []
    for c in range(NCORES):
        idx_all = []
        nid_cols = []
        for k in range(ncells):
            cap = int(caps[k])
            idx, sbv, slot = percell[c][k]
            n = len(idx)
            ia = np.zeros(cap, np.int16)
            ia[:n] = idx.astype(np.int16)
            if n < cap:
                ia[n:] = idx[-1] if n else 0
            idx_all.append(ia)
            sba = np.full(cap, -999, np.int64)
            sla = np.full(cap, -1, np.int64)
            sba[:n] = sbv
            sla[:n] = slot
            for b, s in schedule[k]:
                col = np.full(128, -1.0, np.float16)
                m = sba[b * 128 : (b + 1) * 128] == s
                col[m] = sla[b * 128 : (b + 1) * 128][m].astype(np.float16)
                nid_cols.append(col)
        packed.append(
            (np.concatenate(idx_all), np.stack(nid_cols, axis=1) if nid_cols else None)
        )
    return caps, schedule, packed


def _plan_stage1(src0):
    keys = []
    cell_of = []
    for c in range(NCORES):
        s = np.asarray(src0[c * NODES_PER_CORE : (c + 1) * NODES_PER_CORE])
        nloc = np.repeat(np.arange(s.shape[0]), FANOUT)
        flat = s.reshape(-1)
        w = flat // WINDOW
        sb = nloc // 128
        g = sb // GROUP1
        cell = g * NWIN + w
        order = np.lexsort((flat, sb, cell))
        flat, nloc, w, sb, cell = (
            flat[order], nloc[order], w[order], sb[order], cell[order])
        keys.append(((flat - w * WINDOW).astype(np.int64), sb, nloc - sb * 128))
        cell_of.append(cell)
    return _plan(keys, NG1 * NWIN, cell_of, NSB1)


PHASE_A_ROWS = 0  # stage-2 phase A disabled (0 = all rows in phase B)


def _plan_stage2(src1):
    """Stage-2 rows split into phase A (Q rows < PHASE_A_ROWS, gatherable
    before stage-1's last group finishes) and phase B (the rest). The sb id
    handed to _plan is phase-qualified so each phase gets its own psum
    accumulation stream."""
    flat0 = np.asarray(src1).reshape(-1)  # values in [0, N1)
    seed0 = np.repeat(np.arange(SEEDS), FANOUT)
    keys = []
    cell_of = []
    for c in range(NCORES):
        m = (flat0 // NODES_PER_CORE) == c
        local, seed = flat0[m] % NODES_PER_CORE, seed0[m]
        sb = seed // 128
        phase = (local >= PHASE_A_ROWS).astype(np.int64)
        cell = phase * NG2 + sb // GROUP2
        psb = phase * NSB2 + sb
        order = np.lexsort((local, psb, cell))
        local, seed, sb, psb, cell = (
            local[order], seed[order], sb[order], psb[order], cell[order])
        keys.append((local, psb, seed - sb * 128))
        cell_of.append(cell)
    return _plan(keys, 2 * NG2, cell_of, 2 * NSB2)


def build_kernel(plan1, plan2):
    caps1, sched1, _ = plan1
    caps2, sched2, _ = plan2
    nc = bacc.Bacc(None, target_bir_lowering=False, debug=False)

    tot1 = int(caps1.sum())
    ncol1 = sum(len(s) for s in sched1)
    tot2 = int(caps2.sum())
    ncol2 = sum(len(s) for s in sched2)

    ftab = nc.dram_tensor("ftab", [N0, E1], f16, kind="ExternalInput")
    idx1 = nc.dram_tensor("idx1", [128, tot1 // 16], i16, kind="ExternalInput")
    nid1 = nc.dram_tensor("nid1", [128, ncol1], f16, kind="ExternalInput")
    idx2 = nc.dram_tensor("idx2", [128, tot2 // 16], i16, kind="ExternalInput")
    nid2 = nc.dram_tensor("nid2", [128, ncol2], f16, kind="ExternalInput")
    w1t = nc.dram_tensor("w1t", [128, 4, NH], f16, kind="ExternalInput")  # W1/16 chunks
    b1v = nc.dram_tensor("b1v", [128, 1], f32, kind="ExternalInput")
    w2t = nc.dram_tensor("w2t", [128, 2, NCLS], f16, kind="ExternalInput")  # W2/16
    iot = nc.dram_tensor("iot", [128, 128], f16, kind="ExternalInput")
    partial = nc.dram_tensor("partial", [NSB2 * 128, NCLS], f32, kind="ExternalOutput")

    # total matmuls per (phase-qualified) sb for start/stop flags
    sbtot1 = np.zeros(NSB1, np.int64)
    for s in sched1:
        for _, sb in s:
            sbtot1[sb] += 1
    sbtot2 = np.zeros(2 * NSB2, np.int64)
    for s in sched2:
        for _, sb in s:
            sbtot2[sb] += 1

    # stage-1 per-group idx/nid column counts (for split loads)
    g1_idxcols = [
        int(sum(caps1[g * NWIN + w] for w in range(NWIN))) // 16 for g in range(NG1)
    ]
    g1_nidcols = [
        sum(len(sched1[g * NWIN + w]) for w in range(NWIN)) for g in range(NG1)
    ]

    with TileContext(nc) as tc:
        with (
            tc.tile_pool(name="const", bufs=1) as cpool,
            tc.tile_pool(name="gather", bufs=5) as gpool,
            tc.tile_pool(name="sel", bufs=8) as spool,
            tc.tile_pool(name="epi", bufs=3) as epool,
            tc.tile_pool(name="acc", bufs=NSB2) as apool,
            tc.tile_pool(name="m0psum", bufs=GROUP1, space="PSUM") as mpool,
            tc.tile_pool(name="epipsum", bufs=3, space="PSUM") as eppool,
            tc.tile_pool(name="dram", bufs=1, space="DRAM") as dpool,
        ):
            idx1_g = []
            nid1_g = []
            c0 = 0
            c1 = 0
            for g in range(NG1):
                t = cpool.tile([128, g1_idxcols[g]], i16, name=f"idx1g_{g}")
                nc.sync.dma_start(t[:], idx1[:, c0 : c0 + g1_idxcols[g]])
                idx1_g.append(t)
                c0 += g1_idxcols[g]
                t = cpool.tile([128, g1_nidcols[g]], f16, name=f"nid1g_{g}")
                nc.sync.dma_start(t[:], nid1[:, c1 : c1 + g1_nidcols[g]])
                nid1_g.append(t)
                c1 += g1_nidcols[g]
            idx2_t = cpool.tile([128, tot2 // 16], i16)
            nc.sync.dma_start(idx2_t[:], idx2[:])
            nid2_t = cpool.tile([128, ncol2], f16)
            nc.sync.dma_start(nid2_t[:], nid2[:])
            w1_t = cpool.tile([128, 4, NH], f16)
            nc.sync.dma_start(w1_t[:], w1t[:])
            b1_t = cpool.tile([128, 1], f32)
            nc.sync.dma_start(b1_t[:], b1v[:])
            w2_t = cpool.tile([128, 2, NCLS], f16)
            nc.sync.dma_start(w2_t[:], w2t[:])
            iota_t = cpool.tile([128, 128], f16)
            nc.sync.dma_start(iota_t[:], iot[:])
            ident = cpool.tile([128, 128], f16)
            make_identity(nc, ident[:])

            qtab = dpool.tile([QROWS, E2], f16)

            def gather(g_t, src_view, idx_t, col, n, elem):
                off = 0
                while off < n:
                    m = min(MAXIDX, n - off)
                    nc.gpsimd.dma_gather(
                        out_ap=g_t[:, off // 128 : (off + m) // 128, :],
                        in_ap=src_view,
                        idxs_ap=idx_t[:, col + off // 16 : col + (off + m) // 16],
                        num_idxs=m,
                        num_idxs_reg=m,
                        elem_size=elem,
                    )
                    off += m

            selcnt = [0]

            def build_sel(nid_t, col):
                s_b = spool.tile([128, 128], f16, tag="sb", name="s_b")
                k = selcnt[0]
                selcnt[0] += 1
                if k % 3 == 2:
                    # ACT path: |nid - iota| -> relu(1 - x) == one-hot
                    tmp = spool.tile([128, 128], f16, tag="sbt", name="tmp")
                    nc.scalar.activation(
                        tmp[:], iota_t[:], mybir.ActivationFunctionType.Abs,
                        bias=nid_t[:, col : col + 1], scale=-1.0,
                    )
                    nc.scalar.activation(
                        s_b[:], tmp[:], mybir.ActivationFunctionType.Relu,
                        bias=1.0, scale=-1.0,
                    )
                else:
                    nc.vector.tensor_tensor(
                        out=s_b[:],
                        in0=nid_t[:, col : col + 1].to_broadcast([128, 128]),
                        in1=iota_t[:],
                        op=mybir.AluOpType.is_equal,
                    )
                return s_b

            qgrp = {}
            qgrp_done = collections.defaultdict(int)

            def epilogue(sb, p):
                m0_s = epool.tile([128, E1], f16, tag="m0s", name="m0_s")
                nc.scalar.activation(m0_s[:], p[:], mybir.ActivationFunctionType.Copy)
                h1p = eppool.tile([128, 128], f32, tag="ep", name="h1p")
                for k in range(4):
                    tp = eppool.tile([128, 128], f16, tag="ep", name="tp")
                    nc.tensor.transpose(tp[:], m0_s[:, k * 128 : (k + 1) * 128], ident[:])
                    mt = epool.tile([128, 128], f16, tag="mt", name="mt")
                    nc.vector.tensor_copy(mt[:], tp[:])
                    nc.tensor.matmul(
                        out=h1p[:], lhsT=w1_t[:, k, :], rhs=mt[:],
                        start=(k == 0), stop=(k == 3),
                    )
                h1_s = epool.tile([128, 128], f16, tag="h1", name="h1_s")
                r_s = epool.tile([128, 128], f16, tag="r", name="r_s")
                nc.scalar.activation(h1_s[:], h1p[:], mybir.ActivationFunctionType.Identity, bias=b1_t[:, :1])
                nc.scalar.activation(r_s[:], h1p[:], mybir.ActivationFunctionType.Relu, bias=b1_t[:, :1])
                qp = eppool.tile([47, 128], f32, tag="ep", name="qp")
                nc.tensor.matmul(out=qp[:], lhsT=w2_t[:, 0, :], rhs=h1_s[:], start=True, stop=False)
                nc.tensor.matmul(out=qp[:], lhsT=w2_t[:, 1, :], rhs=r_s[:], start=False, stop=True)
                qT_s = epool.tile([47, 128], f16, tag="qT", name="qT_s")
                nc.vector.tensor_copy(qT_s[:], qp[:])
                q2p = eppool.tile([128, 128], f16, tag="ep", name="q2p")
                nc.tensor.transpose(q2p[:, :47], qT_s[:], ident[:47, :47])
                g = sb // GROUP1
                j = sb % GROUP1
                if g not in qgrp:
                    qgrp[g] = epool.tile([128, GROUP1, E2], f16, tag="qg", name=f"qg_{g}", bufs=2)
                nc.vector.tensor_copy(qgrp[g][:, j, :47], q2p[:, :47])
                nc.vector.memset(qgrp[g][:, j, 47:], 0.0)
                qgrp_done[g] += 1
                if qgrp_done[g] == min(GROUP1, NSB1 - g * GROUP1):
                    t = qgrp.pop(g)
                    nn = qgrp_done[g]
                    nc.sync.dma_start(
                        qtab[g * GROUP1 * 128 : g * GROUP1 * 128 + nn * 128, :].rearrange(
                            "(j p) e -> p j e", p=128
                        ),
                        t[:, :nn, :],
                    )

            # ---- stage 2 machinery (emitted in two phases) ----
            s2_state = {"col": 0, "mcol": 0}
            pp = {}
            accA = {}
            pgrp = {}
            pgrp_done = collections.defaultdict(int)
            sb2_seq = np.zeros(2 * NSB2, np.int64)

            def emit_stage2(cells, src_view):
                for cell in cells:
                    n = int(caps2[cell])
                    if n == 0:
                        continue
                    g_t = gpool.tile([128, n // 128, E2], f16, tag="g2", name="g_t2")
                    gather(g_t, src_view, idx2_t, s2_state["col"], n, E2)
                    s2_state["col"] += n // 16
                    for b, psb in sched2[cell]:
                        if psb not in pp:
                            pp[psb] = eppool.tile([128, 128], f32, tag="ep", name=f"pp_{psb}")
                        s_b = build_sel(nid2_t, s2_state["mcol"])
                        s2_state["mcol"] += 1
                        nc.tensor.matmul(
                            out=pp[psb][:],
                            lhsT=s_b[:],
                            rhs=g_t[:, b, :],
                            start=(sb2_seq[psb] == 0),
                            stop=(sb2_seq[psb] == sbtot2[psb] - 1),
                        )
                        sb2_seq[psb] += 1
                        if sb2_seq[psb] == sbtot2[psb]:
                            p = pp.pop(psb)
                            sb = psb % NSB2
                            if psb < NSB2:
                                a = apool.tile([128, NCLS], f32, tag="accA", name=f"accA_{sb}")
                                nc.vector.tensor_copy(a[:], p[:, :NCLS])
                                accA[sb] = a
                            else:
                                gg = sb // GROUP2
                                jj = sb % GROUP2
                                if gg not in pgrp:
                                    pgrp[gg] = epool.tile(
                                        [128, GROUP2, NCLS], f32, tag="pg", name=f"pg_{gg}", bufs=2
                                    )
                                if sb in accA:
                                    nc.vector.tensor_tensor(
                                        out=pgrp[gg][:, jj, :],
                                        in0=p[:, :NCLS],
                                        in1=accA.pop(sb)[:],
                                        op=mybir.AluOpType.add,
                                    )
                                else:
                                    nc.vector.tensor_copy(pgrp[gg][:, jj, :], p[:, :NCLS])
                                pgrp_done[gg] += 1
                                if pgrp_done[gg] == min(GROUP2, NSB2 - gg * GROUP2):
                                    t = pgrp.pop(gg)
                                    nn = pgrp_done[gg]
                                    nc.sync.dma_start(
                                        partial[
                                            gg * GROUP2 * 128 : gg * GROUP2 * 128 + nn * 128, :
                                        ].rearrange("(j p) e -> p j e", p=128),
                                        t[:, :nn, :],
                                    )

            # ---- stage 1 (with stage-2 phase A interleaved into the last
            # group so its qtab[0:PHASE_A_ROWS] dependency is ready) ----
            m0_psum = {}
            sb_seq = np.zeros(NSB1, np.int64)
            g1_cols = [[0, 0] for _ in range(NG1)]  # per-group (col, mcol)

            def emit_s1_cell(g, w):
                cell = g * NWIN + w
                n = int(caps1[cell])
                if n == 0:
                    return
                g_t = gpool.tile([128, n // 128, E1], f16, tag="g1", name="g_t")
                wsz = min(WINDOW, N0 - w * WINDOW)
                gather(g_t, ftab[w * WINDOW : w * WINDOW + wsz, :], idx1_g[g], g1_cols[g][0], n, E1)
                g1_cols[g][0] += n // 16
                for b, sb in sched1[cell]:
                    if sb not in m0_psum:
                        m0_psum[sb] = mpool.tile([128, E1], f32, tag="m0", name=f"m0_{sb}")
                    s_b = build_sel(nid1_g[g], g1_cols[g][1])
                    g1_cols[g][1] += 1
                    nc.tensor.matmul(
                        out=m0_psum[sb][:],
                        lhsT=s_b[:],
                        rhs=g_t[:, b, :],
                        start=(sb_seq[sb] == 0),
                        stop=(sb_seq[sb] == sbtot1[sb] - 1),
                    )
                    sb_seq[sb] += 1
                    if sb_seq[sb] == sbtot1[sb]:
                        epilogue(sb, m0_psum.pop(sb))

            for g in range(NG1 - 1):
                for w in range(NWIN):
                    emit_s1_cell(g, w)
            # last group: interleave phase-A stage-2 cells after the first
            # two feature cells
            acell = 0
            for w in range(NWIN):
                emit_s1_cell(NG1 - 1, w)
                if w >= 1 and acell < NG2:
                    emit_stage2([acell, acell + 1] if acell + 1 < NG2 else [acell],
                                qtab[0:PHASE_A_ROWS, :])
                    acell += 2
            while acell < NG2:
                emit_stage2([acell], qtab[0:PHASE_A_ROWS, :])
                acell += 1
            # phase B of stage 2
            emit_stage2(range(NG2, 2 * NG2), qtab[:])
    nc.compile()
    return nc
    return nc


def _host_inputs(features, src0, src1, W1, b1, W2):
    plan1 = _plan_stage1(src0)
    plan2 = _plan_stage2(src1)

    ftab_np = np.zeros((N0, E1), np.float16)
    ftab_np[:, :IN_F] = np.asarray(features, np.float32).astype(np.float16)

    w1_np = np.zeros((128, 4, NH), np.float16)
    w1f = np.zeros((E1, NH), np.float32)
    w1f[:IN_F] = np.asarray(W1, np.float32) / FANOUT
    for k in range(4):
        w1_np[:, k, :] = w1f[k * 128 : (k + 1) * 128].astype(np.float16)
    b1_np = np.asarray(b1, np.float32).reshape(128, 1)
    w2_np = np.zeros((128, 2, NCLS), np.float16)
    w2f = np.asarray(W2, np.float32) / FANOUT
    w2_np[:, 0, :] = w2f[:NH].astype(np.float16)
    w2_np[:, 1, :] = w2f[NH:].astype(np.float16)
    iota_np = np.tile(np.arange(128, dtype=np.float16), (128, 1))

    in_maps = []
    for c in range(NCORES):
        idx1c, nid1c = plan1[2][c]
        idx2c, nid2c = plan2[2][c]
        in_maps.append(
            {
                "ftab": ftab_np,
                "idx1": np.ascontiguousarray(_wrap_idxs(idx1c)),
                "nid1": np.ascontiguousarray(nid1c),
                "idx2": np.ascontiguousarray(_wrap_idxs(idx2c)),
                "nid2": np.ascontiguousarray(nid2c),
                "w1t": w1_np,
                "b1v": b1_np,
                "w2t": w2_np,
                "iot": iota_np,
            }
        )
    return plan1, plan2, in_maps


_cache = {}


def kernel(features, src0, src1, W1, b1, W2, b2):
    plan1, plan2, in_maps = _host_inputs(features, src0, src1, W1, b1, W2)
    key = (plan1[0].tobytes(), plan2[0].tobytes(),
           str(plan1[1]).encode(), str(plan2[1]).encode())
    import hashlib
    key = hashlib.sha256(b"|".join(key)).hexdigest()
    if key not in _cache:
        _cache[key] = build_kernel(plan1, plan2)
    nc = _cache[key]
    res = run_bass_kernel_spmd(nc, in_maps, core_ids=list(range(NCORES)))
    out = np.zeros((SEEDS, NCLS), np.float64)
    for c in range(NCORES):
        out += res.results[c]["partial"][:SEEDS].astype(np.float64)
    out = out + np.asarray(b2, np.float64)[None, :]
    return out.astype(np.float32)


if __name__ == "__main__":
    rng = np.random.default_rng(0)
    feats = rng.standard_normal((N0, IN_F), dtype=np.float32)
    src0 = rng.integers(0, N0, size=(N1, FANOUT))
    src1 = rng.integers(0, N1, size=(N2, FANOUT))
    W1 = rng.standard_normal((IN_F, NH), dtype=np.float32) * 0.05
    b1 = np.zeros(NH, np.float32)
    W2 = rng.standard_normal((2 * NH, NCLS), dtype=np.float32) * 0.05
    b2 = np.zeros(NCLS, np.float32)
    out = kernel(feats, src0, src1, W1, b1, W2, b2)
    m0 = feats[src0].mean(axis=1)
    h1 = m0 @ W1 + b1
    h1 = np.concatenate([h1, np.maximum(h1, 0)], axis=1)
    m1 = h1[src1].mean(axis=1)
    ref = m1 @ W2 + b2
    rel = np.abs(out - ref) / (np.abs(ref) + 1e-5)
    print("max rel err:", rel.max(), "mean:", rel.mean())
    print("norm rel:", np.linalg.norm(out - ref) / np.linalg.norm(ref))

